# revision 1
# baseline (speedup 1.0000x reference)
"""Trainium2 Bass kernel for BayesianLinear sampling (B=2048, OUT=IN=256).

out[b,o] = sum_i (mu[o,i] + std[o,i]*eps_w[b,o,i]) * x[b,i]
         + bias_mu[o] + bias_std[o]*eps_b[b,o]

Data-parallel over batch across 8 NeuronCores (256 samples each, no
collectives needed for the forward pass).  Memory-bound: eps_w is 512MB
fp32; each core streams its 64MB shard from HBM exactly once, cast to
fp16 in-flight by the SWDGE DMA engines (halves SBUF traffic and enables
the DVE 2x fp16 perf mode).  Layout: o on partitions (2 tiles of 128),
i contiguous on the free dim.  Per sample b:
  TT  (DVE, 2x fp16):  Q = eps_f16 * std_f16          (batched 16 samples)
  PE  rank-1 matmuls:  x_bc[o,i] = x[b,i]   (ones-outer-product broadcast
                       along partitions, 2 samples per N=512 matmul)
  ACT copy:            x_bc PSUM -> SBUF fp16
  STT (DVE, fused):    out_col[o] = sum_i Q[o,i]*x_bc[o,i]
                       (scalar_tensor_tensor with accum_out: multiply and
                       free-axis reduce in one 1x DVE pass; the fused
                       tensor_tensor_reduce op is broken on this runtime)
base = x@mu.T + bias_mu + bias_std*eps_b^T is computed once on TensorE +
DVE and added at the end; the (o,b) result is transposed to (b,o) via PE
transpose and stored contiguously.

Engine budget per core (cost model ~267us): DMA ~180us (the HBM floor for
64MB fp32), DVE ~245us busy (70us fold + 167us fused reduce -- the
reduce's 1x mode and (128,1) accum granularity are the binding limits),
ACT ~75us, PE ~30us.

All matmul operands are packed into two constants tensors (one f32, one
f16) loaded by a single DMA each, so every PE instruction sees its
constants through one DMA semaphore (the walrus build here allows only
one sync-wait per compute instruction; Bacc.compile()'s
generate_event_semaphores pass splits the rest).
"""

import sys

sys.path.insert(0, "/opt/trn_rl_repo")

import numpy as np

import concourse.bass as bass
import concourse.bacc as bacc
import concourse.mybir as mybir
from concourse import tile
from concourse.bass_utils import run_bass_kernel_spmd
from concourse.tile_rust import add_dep_helper

N_CORES = 8
B, OUT, IN = 2048, 256, 256
B_CORE = B // N_CORES          # 256 samples per core
GE = 16                        # samples per eps DMA / TT1 instruction
NGE = B_CORE // GE             # 16 eps groups
G = 4                          # samples per PSUM x_bc tile (2 banks)
F32 = mybir.dt.float32
F16 = mybir.dt.float16
MULT = mybir.AluOpType.mult
ADD = mybir.AluOpType.add

# f32 constants pack: [muT0 | xT0 | muT1 | xT1 | ident]  (128, 1152)
C32_W = 4 * 256 + 128
# f16 constants pack: [srep0 | srep1 | xquad | ones-row]  (128, 32896)
SREP_W = GE * IN               # 8192 per o-tile
XQ_W = B_CORE * IN // 4        # 16384 per quad row
C16_W = 2 * SREP_W + XQ_W + 128
XQ_OFF = 2 * SREP_W
ONES_OFF = XQ_OFF + XQ_W


def _build_nc(variant="nofence"):
    FENCES = variant not in ("nofence", "noxbc", "noquad", "noloop", "hybrid")
    # In the hybrid variant, 3 of every 8 PSUM sub-groups route their
    # per-sample reduction through ScalarE (DVE does the 2x fp16 multiply,
    # ACT does the free-axis accumulate), balancing the two engines.
    HYBRID = variant == "hybrid"
    nc = bacc.Bacc(trn_type="TRN2")

    eps = nc.declare_dram_parameter("eps", [B_CORE, OUT, IN], F32, isOutput=False)
    c32 = nc.declare_dram_parameter("c32", [128, C32_W], F32, isOutput=False)
    c16 = nc.declare_dram_parameter("c16", [128, C16_W], F16, isOutput=False)
    # aux pack: [epsbT0 | epsbT1 | bmu0 | bmu1 | bstd0 | bstd1]
    aux = nc.declare_dram_parameter("aux", [128, 2 * B_CORE + 4], F32,
                                    isOutput=False)
    out = nc.declare_dram_parameter("out", [B_CORE, OUT], F32, isOutput=True)

    eps3 = eps.rearrange("b o i -> o b i")  # partition=o view for DMA

    with tile.TileContext(nc) as tc:
        with tc.tile_pool(name="const", bufs=1) as cpool:
            c32_sb = cpool.tile([128, C32_W], F32, tag="c32", name="c32sb")
            c16_sb = cpool.tile([128, C16_W], F16, tag="c16", name="c16sb")
            aux_sb = cpool.tile([128, 2 * B_CORE + 4], F32, tag="aux",
                                name="auxsb")
            base = [cpool.tile([128, B_CORE], F32, tag=f"base{t}", name=f"base{t}")
                    for t in range(2)]
            outsb = [cpool.tile([128, B_CORE], F32, tag=f"outsb{t}", name=f"outsb{t}")
                     for t in range(2)]
            tiny = cpool.tile([128, 2], F32, tag="tiny", name="tiny")
            actscr = cpool.tile([128, 1], F32, tag="actscr", name="actscr")
            gscr = cpool.tile([128, 1], F32, tag="gscr", name="gscr")
            obo = [cpool.tile([128, OUT], F32, tag=f"obo{h}", name=f"obo{h}")
                   for h in range(2)]

            nc.sync.dma_start(out=c32_sb[:], in_=c32[:])
            nc.sync.dma_start(out=c16_sb[:], in_=c16[:])
            nc.sync.dma_start(out=aux_sb[:], in_=aux[:])

            def epsbT_ap(t):
                return aux_sb[:, t * B_CORE:(t + 1) * B_CORE]

            def bmu_ap(t):
                return aux_sb[:, 2 * B_CORE + t:2 * B_CORE + t + 1]

            def bstd_ap(t):
                return aux_sb[:, 2 * B_CORE + 2 + t:2 * B_CORE + 3 + t]

            def muT_ap(k, t):  # (128, 128) lhsT slice: k-tile k, o-half t
                return c32_sb[:, k * 512 + t * 128: k * 512 + (t + 1) * 128]

            def xT_ap(k):      # (128, B_CORE) rhs slice
                return c32_sb[:, k * 512 + 256: k * 512 + 512]

            ident_ap = c32_sb[:, 1024:1152]

            def ones_ap(q):
                return c16_sb[32 * q:32 * q + 1, ONES_OFF:ONES_OFF + 128]

            def srep_ap(t):
                return c16_sb[:, t * SREP_W:(t + 1) * SREP_W]

            def xq_ap(b0, n):  # x[b0*IN : b0*IN+n] as (1, n) on quad row
                off = b0 * IN
                q, r = divmod(off, XQ_W)
                assert r + n <= XQ_W
                return q, c16_sb[32 * q:32 * q + 1, XQ_OFF + r:XQ_OFF + r + n]

            # ---- prologue: base[t] = mu @ x^T + bias_mu + bias_std*eps_b^T
            with tc.tile_pool(name="pmu", bufs=2, space="PSUM") as pmupool:
                pmus = []
                for t in range(2):
                    pmu = pmupool.tile([128, B_CORE], F32, tag="pmu",
                                       name=f"pmu{t}")
                    nc.tensor.matmul(pmu[:], muT_ap(0, t), xT_ap(0),
                                     start=True, stop=False)
                    nc.tensor.matmul(pmu[:], muT_ap(1, t), xT_ap(1),
                                     start=False, stop=True)
                    pmus.append(pmu)
                # DVE pre-fences: absorb the aux/c16 DMA waits so each compute
                # op below carries at most one cross-engine wait (walrus limit)
                dve_f2 = None
                if FENCES:
                    dve_f1 = nc.vector.tensor_copy(tiny[:, 0:1], aux_sb[:, 0:1])
                    dve_f2 = nc.vector.tensor_copy(
                        tiny[:, 1:2], c16_sb[:, 0:2].bitcast(F32))
                    add_dep_helper(dve_f2.ins, dve_f1.ins, False, "order")
                base_done = None
                for t in range(2):
                    # base = eps_bT*bstd + pmu
                    stt = nc.vector.scalar_tensor_tensor(
                        out=base[t][:], in0=epsbT_ap(t), scalar=bstd_ap(t),
                        in1=pmus[t][:], op0=MULT, op1=ADD,
                    )
                    if FENCES and t == 0:
                        add_dep_helper(stt.ins, dve_f2.ins, False,
                                       "order after DVE fences")
                    # base += bias_mu
                    base_done = nc.vector.tensor_scalar_add(
                        base[t][:], base[t][:], bmu_ap(t))
                if FENCES:
                    # ACT init fence: observe DVE through end of prologue
                    act_init = nc.scalar.copy(out=actscr[:, 0:1],
                                              in_=tiny[:, 0:1])
                    add_dep_helper(act_init.ins, base_done.ins, True,
                                   "ACT observes prologue DVE")

            # ---- main loop over eps groups
            with (
                tc.tile_pool(name="eg", bufs=4) as egpool,
                tc.tile_pool(name="qg", bufs=2) as qpool,
                tc.tile_pool(name="xbcp", bufs=3, space="PSUM") as xbcpool,
                tc.tile_pool(name="xbcs", bufs=4) as xbcsbpool,
                tc.tile_pool(name="scr", bufs=4) as scrpool,
                tc.tile_pool(name="rg", bufs=2) as rpool,
            ):
                xbc_copies = []  # ACT copies that release recycled PSUM bufs
                group_last_ttr = []  # last TTR reading each group's xbc_sb
                group_tt1 = []       # the two TT1s of each eps group
                for ge in range(NGE):
                    # GpSimd fence: the POOL sequencer observes the DVE TT1s
                    # that last read the eg slots being recycled, so the eps
                    # DMAs below carry only their DMA-lane wait.
                    gfence = None
                    if FENCES and ge >= 4:
                        gfence = nc.gpsimd.memset(gscr[:, 0:1], 0.0)
                        for tt in group_tt1[ge - 4]:
                            add_dep_helper(gfence.ins, tt.ins, True,
                                           "absorb eg recycle wait")
                    eg, qg = [], []
                    for t in range(2):
                        e = egpool.tile([128, GE * IN], F16, tag=f"eg{t}",
                                        name=f"eg{t}")
                        dma = nc.gpsimd.dma_start(
                            out=e[:].rearrange("p (g i) -> p g i", i=IN),
                            in_=eps3[t * 128:(t + 1) * 128, ge * GE:(ge + 1) * GE, :],
                        )
                        if gfence is not None:
                            add_dep_helper(dma.ins, gfence.ins, False,
                                           "order after gpsimd fence")
                            gfence = None
                        eg.append(e)
                    for t in range(2):
                        q = qpool.tile([128, GE * IN], F16, tag=f"qg{t}",
                                       name=f"qg{t}")
                        tt1 = nc.vector.tensor_tensor(
                            out=q[:], in0=eg[t][:], in1=srep_ap(t), op=MULT
                        )
                        if FENCES and ge == 0:
                            add_dep_helper(tt1.ins, dve_f2.ins, False,
                                           "c16 observed via DVE fences")
                        qg.append(q)
                        if t == 0:
                            group_tt1.append([tt1])
                        else:
                            group_tt1[-1].append(tt1)

                    for c in range(GE // G):  # PSUM-sized sub-groups of G samples
                        # PE fence: absorb the PSUM-recycle (WAR vs ACT copy)
                        # wait so the matmuls below carry <=1 sync wait each
                        # (walrus allows only one wait on the PE LW slot).
                        n = len(xbc_copies)
                        fence = None
                        if FENCES and n >= 3:
                            fence = nc.tensor.ldweights(ones_ap(0))
                            add_dep_helper(fence.ins, xbc_copies[n - 3].ins,
                                           True, "absorb psum WAR wait")
                        elif FENCES and n == 0:
                            # pmu pool banks recycled into xbc pool: absorb
                            # the DVE WAR (base computation read pmu last).
                            # Read c32 (already PE-observed via mu matmuls)
                            # so this fence carries only the DVE wait.
                            fence = nc.tensor.ldweights(
                                c32_sb[:, 1024:1026].bitcast(F16))
                            add_dep_helper(fence.ins, base_done.ins,
                                           True, "absorb pmu->xbc WAR wait")
                        xbc_ps = xbcpool.tile([128, G * IN], F32, tag="xbc")
                        for j in range(G * IN // 512):
                            b0 = ge * GE + c * G + j * (512 // IN)
                            q, xap = xq_ap(b0, 512)
                            if variant == "noquad":
                                q = 0
                                xap = c16_sb[0:1, XQ_OFF:XQ_OFF + 512]
                                tp = None
                            else:
                                tp = (32 * q, 0)
                            mm = nc.tensor.matmul(
                                xbc_ps[:, j * 512:(j + 1) * 512],
                                ones_ap(q), xap,
                                start=True, stop=True,
                                tile_position=tp,
                            )
                            if fence is not None:
                                add_dep_helper(mm.ins, fence.ins, False,
                                               "order after fence")
                                fence = None
                        xbc_sb = xbcsbpool.tile([128, G * IN], F16, tag="xbcs")
                        if FENCES and n >= 3:
                            # ACT fence: absorb the DVE wait (xbc_sb slot
                            # recycle: last reader was a TTR 3 groups ago)
                            af = nc.scalar.copy(out=actscr[:, 0:1],
                                                in_=tiny[:, 0:1])
                            add_dep_helper(af.ins, group_last_ttr[n - 3].ins,
                                           True, "absorb xbcs recycle wait")
                            cp = nc.scalar.copy(out=xbc_sb[:], in_=xbc_ps[:])
                            add_dep_helper(cp.ins, af.ins, False,
                                           "order after ACT fence")
                        else:
                            cp = nc.scalar.copy(out=xbc_sb[:], in_=xbc_ps[:])
                            if FENCES and n == 0:
                                add_dep_helper(cp.ins, act_init.ins, False,
                                               "order after ACT init fence")
                        xbc_copies.append(cp)

                        sg = ge * (GE // G) + c     # sub-group index 0..63
                        act_path = HYBRID and (sg % 8) < 3
                        if act_path:
                            # DVE: one batched 2x multiply per o-tile;
                            # ACT: per-sample accumulate into out columns.
                            for t in range(2):
                                r = rpool.tile([128, G * IN], F16,
                                               tag=f"rg{t}", name=f"rg{t}")
                                last_ttr = nc.vector.tensor_tensor(
                                    out=r[:],
                                    in0=qg[t][:, c * G * IN:(c + 1) * G * IN],
                                    in1=xbc_sb[:], op=MULT,
                                )
                                for j in range(G):
                                    b = ge * GE + c * G + j
                                    ascr = rpool.tile([128, IN], F16,
                                                      tag="ascr", name="ascr")
                                    nc.scalar.activation(
                                        out=ascr[:],
                                        in_=r[:, j * IN:(j + 1) * IN],
                                        func=mybir.ActivationFunctionType.Copy,
                                        accum_out=outsb[t][:, b:b + 1],
                                    )
                        else:
                            for t in range(2):
                                for j in range(G):
                                    b = ge * GE + c * G + j
                                    scr = scrpool.tile([128, IN], F16,
                                                       tag="scr")
                                    last_ttr = nc.vector.scalar_tensor_tensor(
                                        out=scr[:],
                                        in0=qg[t][:, (c * G + j) * IN:(c * G + j + 1) * IN],
                                        scalar=1.0,
                                        in1=xbc_sb[:, j * IN:(j + 1) * IN],
                                        op0=MULT, op1=MULT,
                                        accum_out=outsb[t][:, b:b + 1],
                                    )
                        group_last_ttr.append(last_ttr)

            # ---- epilogue: add base, transpose (o,b) -> (b,o), store
            for t in range(2):
                nc.vector.tensor_tensor(out=outsb[t][:], in0=outsb[t][:],
                                        in1=base[t][:], op=ADD)
            with tc.tile_pool(name="ptr", bufs=4, space="PSUM") as ptrpool:
                # absorb the xbc->ptr PSUM transition waits (ACT copy WAR)
                # and the outsb DVE dependency before the transposes
                if FENCES:
                    fence_a = nc.tensor.ldweights(ones_ap(0))
                    add_dep_helper(fence_a.ins, xbc_copies[-1].ins,
                                   True, "absorb xbc->ptr WAR wait")
                    fence_b = nc.tensor.ldweights(ones_ap(0))
                    add_dep_helper(fence_b.ins, last_ttr.ins,
                                   True, "absorb outsb DVE wait")
                    add_dep_helper(fence_b.ins, fence_a.ins, False, "order")
                first_tr = None
                for t in range(2):
                    for h in range(2):
                        ptr = ptrpool.tile([128, 128], F32, tag="ptr")
                        tr = nc.tensor.transpose(
                            ptr[:], outsb[t][:, h * 128:(h + 1) * 128], ident_ap
                        )
                        if FENCES and first_tr is None:
                            first_tr = tr
                            add_dep_helper(tr.ins, fence_b.ins, False,
                                           "order after fences")
                        nc.vector.tensor_copy(
                            obo[h][:, t * 128:(t + 1) * 128], ptr[:])
                for h in range(2):
                    nc.sync.dma_start(
                        out=out[h * 128:(h + 1) * 128, :], in_=obo[h][:]
                    )

    nc.compile()
    return nc


_NC_CACHE = None


def _get_nc():
    global _NC_CACHE
    if _NC_CACHE is None:
        _NC_CACHE = _build_nc()
    return _NC_CACHE


def _prep_inputs(x, weight_mu, weight_logvar, bias_mu, bias_logvar, eps_w, eps_b):
    """Host-side prep: shard big tensors over batch, precompute small params."""
    x = np.asarray(x, np.float32)
    weight_mu = np.asarray(weight_mu, np.float32)
    weight_logvar = np.asarray(weight_logvar, np.float32)
    bias_mu = np.asarray(bias_mu, np.float32)
    bias_logvar = np.asarray(bias_logvar, np.float32)
    eps_w = np.asarray(eps_w, np.float32)
    eps_b = np.asarray(eps_b, np.float32)

    s16 = np.exp(0.5 * weight_logvar).astype(np.float16)     # (OUT, IN)
    muT = np.ascontiguousarray(weight_mu.T)                  # (IN, OUT)
    bstd = np.exp(0.5 * bias_logvar).astype(np.float32)
    bmu2 = np.ascontiguousarray(bias_mu.reshape(2, 128, 1))
    bstd2 = np.ascontiguousarray(bstd.reshape(2, 128, 1))

    in_maps = []
    for c in range(N_CORES):
        sl = slice(c * B_CORE, (c + 1) * B_CORE)
        x_c = x[sl]                                          # (B_CORE, IN)
        xT_c = np.ascontiguousarray(x_c.T)                   # (IN, B_CORE)

        c32 = np.zeros((128, C32_W), np.float32)
        for k in range(2):
            c32[:, k * 512:k * 512 + 256] = muT[k * 128:(k + 1) * 128, :]
            c32[:, k * 512 + 256:k * 512 + 512] = xT_c[k * 128:(k + 1) * 128, :]
        c32[:, 1024:1152] = np.eye(128, dtype=np.float32)

        c16 = np.zeros((128, C16_W), np.float16)
        for t in range(2):
            c16[:, t * SREP_W:(t + 1) * SREP_W] = np.tile(
                s16[t * 128:(t + 1) * 128, :], (1, GE))
        xf = x_c.reshape(4, XQ_W).astype(np.float16)
        for q in range(4):
            c16[32 * q, XQ_OFF:XQ_OFF + XQ_W] = xf[q]
        for q in range(4):
            c16[32 * q, ONES_OFF:ONES_OFF + 128] = np.float16(1.0)

        auxm = np.zeros((128, 2 * B_CORE + 4), np.float32)
        ebT = eps_b[sl].T.reshape(2, 128, B_CORE)
        auxm[:, 0:B_CORE] = ebT[0]
        auxm[:, B_CORE:2 * B_CORE] = ebT[1]
        auxm[:, 2 * B_CORE + 0] = bmu2[0, :, 0]
        auxm[:, 2 * B_CORE + 1] = bmu2[1, :, 0]
        auxm[:, 2 * B_CORE + 2] = bstd2[0, :, 0]
        auxm[:, 2 * B_CORE + 3] = bstd2[1, :, 0]

        in_maps.append({
            "eps": eps_w[sl],
            "c32": c32,
            "c16": c16,
            "aux": auxm,
        })
    return in_maps


def run(trace=False, **inputs):
    nc = _get_nc()
    in_maps = _prep_inputs(**inputs)
    res = run_bass_kernel_spmd(nc, in_maps, list(range(N_CORES)), trace=trace)
    out = np.concatenate([np.asarray(res.results[c]["out"]) for c in range(N_CORES)],
                         axis=0)
    return out.astype(np.float32), res


def kernel(**inputs) -> np.ndarray:
    out, _ = run(trace=False, **inputs)
    return out



# revision 2
# speedup vs baseline: 2.2274x; 2.2274x over previous
"""Trainium2 Bass kernel for BayesianLinear sampling (B=2048, OUT=IN=256).

out[b,o] = sum_i (mu[o,i] + std[o,i]*eps_w[b,o,i]) * x[b,i]
         + bias_mu[o] + bias_std[o]*eps_b[b,o]

Data-parallel over batch across 8 NeuronCores (256 samples each).

Per-core pipeline (out is produced transposed as (o, b); the host
transposes back):

  eps load   -- split across three DMA queues so the transfers overlap:
               even 16-sample chunks stream through the gpsimd (SWDGE)
               queue with an in-flight f32->f16 cast; odd chunks come in
               as raw f32 via the SP and ACT HWDGE queues (one o-half
               each) and are cast to f16 by the Scalar engine.
  PE         -- per 128x128 block, transpose eps (o,i)->(i,o) into an
               f16 PSUM slab (is_transpose matmul against an f16
               identity, 1 cycle/row).
  DVE        -- one tensor_tensor per 4-sample group: Q = epsT * stdT
               (PSUM f16 -> SBUF f16, 2x perf mode).  This doubles as
               the PSUM->SBUF copy.
  PE         -- per (sample, i-half, o-half): n=1 matmul with the Q
               block as the stationary operand and the sample's x
               column as the moving operand, accumulating
               outT[o, b] = sum_i Q[i,o] x[b,i] into PSUM.  The
               mu @ x^T base term is 4 more matmuls into a second PSUM
               tile.
  epilogue   -- DVE adds base + (bias_mu + bias_std*eps_b)^T (host
               precomputed) and the result is DMA'd out as (o, b).

The small parameters (mu, std=exp(0.5*logvar), x, biases) are packed on
the host; eps_w is shipped to the device untouched.
"""

import sys

sys.path.insert(0, "/opt/trn_rl_repo")

import numpy as np

import concourse.bass as bass
import concourse.bacc as bacc
import concourse.mybir as mybir
from concourse import tile
from concourse.bass_utils import run_bass_kernel_spmd

N_CORES = 8
B, OUT, IN = 2048, 256, 256
B_CORE = B // N_CORES          # 256 samples per core
SD = 16                        # samples per DMA chunk
S = 4                          # samples per transpose/fold group
NCHUNK = B_CORE // SD          # 16 chunks
F32 = mybir.dt.float32
F16 = mybir.dt.float16
MULT = mybir.AluOpType.mult
ADD = mybir.AluOpType.add

SREP_W = S * 2 * 2 * 128       # 2048: (h_i, s, h_o) blocks of 128


def _blk(h_i, s, h_o):
    """Free-dim offset of block (h_i, s, h_o) in a Q / QT slab."""
    return ((h_i * S + s) * 2 + h_o) * 128


def _build_nc():
    nc = bacc.Bacc(trn_type="TRN2")

    eps = nc.declare_dram_parameter("eps", [B_CORE, OUT, IN], F32, isOutput=False)
    srepT = nc.declare_dram_parameter("srepT", [128, SREP_W], F16, isOutput=False)
    xT = nc.declare_dram_parameter("xT", [128, 2 * B_CORE], F16, isOutput=False)
    muT = nc.declare_dram_parameter("muT", [128, 2 * OUT], F16, isOutput=False)
    biasesT = nc.declare_dram_parameter("biasesT", [128, 2 * B_CORE], F32,
                                        isOutput=False)
    ident = nc.declare_dram_parameter("ident", [128, 128], F16, isOutput=False)
    out = nc.declare_dram_parameter("out", [OUT, B_CORE], F32, isOutput=True)

    eps3 = eps.rearrange("b o i -> o b i")  # partition = o view for DMA

    with tile.TileContext(nc) as tc:
        with tc.tile_pool(name="const", bufs=1) as cpool:
            srepT_sb = cpool.tile([128, SREP_W], F16, tag="srepT", name="srepTsb")
            xT_sb = cpool.tile([128, 2 * B_CORE], F16, tag="xT", name="xTsb")
            muT_sb = cpool.tile([128, 2 * OUT], F16, tag="muT", name="muTsb")
            bias_sb = cpool.tile([128, 2 * B_CORE], F32, tag="biasesT",
                                 name="biassb")
            id_sb = cpool.tile([128, 128], F16, tag="ident", name="idsb")
            out_sb = [cpool.tile([128, B_CORE], F32, tag=f"osb{h}",
                                 name=f"osb{h}") for h in range(2)]
            tmp_sb = [cpool.tile([128, B_CORE], F32, tag=f"tsb{h}",
                                 name=f"tsb{h}") for h in range(2)]

            nc.sync.dma_start(out=srepT_sb[:], in_=srepT[:])
            nc.sync.dma_start(out=xT_sb[:], in_=xT[:])
            nc.sync.dma_start(out=muT_sb[:], in_=muT[:])
            nc.sync.dma_start(out=bias_sb[:], in_=biasesT[:])
            nc.sync.dma_start(out=id_sb[:], in_=ident[:])

            with (
                tc.tile_pool(name="ef", bufs=3) as efpool,
                tc.tile_pool(name="e32", bufs=2) as e32pool,
                tc.tile_pool(name="qt", bufs=2, space="PSUM") as qtpool,
                tc.tile_pool(name="q", bufs=3) as qpool,
                tc.tile_pool(name="acc", bufs=1, space="PSUM") as accpool,
            ):
                acc = [accpool.tile([128, B_CORE], F32, tag=f"acc{h}",
                                    name=f"acc{h}") for h in range(2)]
                pbase = [accpool.tile([128, B_CORE], F32, tag=f"pb{h}",
                                      name=f"pb{h}") for h in range(2)]

                # base term: pbase[h_o][o_l, b] = sum_i mu[o,i] x[b,i]
                for h_o in range(2):
                    for h_i in range(2):
                        nc.tensor.matmul(
                            pbase[h_o][:],
                            muT_sb[:, h_i * OUT + h_o * 128:
                                   h_i * OUT + (h_o + 1) * 128],
                            xT_sb[:, h_i * B_CORE:(h_i + 1) * B_CORE],
                            start=(h_i == 0), stop=(h_i == 1),
                        )

                for c in range(NCHUNK):
                    ef = efpool.tile([128, 2 * SD * IN], F16, tag="ef",
                                     name="ef")
                    if c % 2 == 0:
                        # SWDGE queue: in-flight f32 -> f16 cast
                        for h_o in range(2):
                            nc.gpsimd.dma_start(
                                out=ef[:, h_o * SD * IN:(h_o + 1) * SD * IN]
                                .rearrange("p (s i) -> p s i", i=IN),
                                in_=eps3[h_o * 128:(h_o + 1) * 128,
                                         c * SD:(c + 1) * SD, :],
                            )
                    else:
                        # HWDGE queues (SP + ACT): raw f32, then ACT casts
                        e32 = e32pool.tile([128, 2 * SD * IN], F32, tag="e32",
                                           name="e32")
                        for h_o, eng in ((0, nc.sync), (1, nc.scalar)):
                            eng.dma_start(
                                out=e32[:, h_o * SD * IN:(h_o + 1) * SD * IN]
                                .rearrange("p (s i) -> p s i", i=IN),
                                in_=eps3[h_o * 128:(h_o + 1) * 128,
                                         c * SD:(c + 1) * SD, :],
                            )
                        for h_o in range(2):
                            nc.scalar.copy(
                                out=ef[:, h_o * SD * IN:(h_o + 1) * SD * IN],
                                in_=e32[:, h_o * SD * IN:(h_o + 1) * SD * IN],
                            )

                    for gs in range(SD // S):
                        qt = qtpool.tile([128, SREP_W], F16, tag="qt",
                                         name="qt")
                        for h_o in range(2):
                            for s in range(S):
                                for h_i in range(2):
                                    nc.tensor.transpose(
                                        qt[:, _blk(h_i, s, h_o):
                                           _blk(h_i, s, h_o) + 128],
                                        ef[:, (h_o * SD + gs * S + s) * IN
                                           + h_i * 128:
                                           (h_o * SD + gs * S + s) * IN
                                           + h_i * 128 + 128],
                                        id_sb[:],
                                    )
                        q = qpool.tile([128, SREP_W], F16, tag="q", name="q")
                        nc.vector.tensor_tensor(out=q[:], in0=qt[:],
                                                in1=srepT_sb[:], op=MULT)
                        for s in range(S):
                            b = c * SD + gs * S + s
                            for h_o in range(2):
                                for h_i in range(2):
                                    nc.tensor.matmul(
                                        acc[h_o][:, b:b + 1],
                                        q[:, _blk(h_i, s, h_o):
                                          _blk(h_i, s, h_o) + 128],
                                        xT_sb[:, h_i * B_CORE + b:
                                              h_i * B_CORE + b + 1],
                                        start=(h_i == 0), stop=(h_i == 1),
                                    )

                # epilogue: out_sb = acc + biasesT + pbase  (each DVE op
                # touches at most one PSUM operand)
                for h_o in range(2):
                    nc.vector.tensor_tensor(
                        out=tmp_sb[h_o][:], in0=acc[h_o][:],
                        in1=bias_sb[:, h_o * B_CORE:(h_o + 1) * B_CORE],
                        op=ADD)
                    nc.vector.tensor_tensor(
                        out=out_sb[h_o][:], in0=pbase[h_o][:],
                        in1=tmp_sb[h_o][:], op=ADD)
                    nc.sync.dma_start(
                        out=out[h_o * 128:(h_o + 1) * 128, :],
                        in_=out_sb[h_o][:])

    nc.compile()
    return nc


_NC_CACHE = None


def _get_nc():
    global _NC_CACHE
    if _NC_CACHE is None:
        _NC_CACHE = _build_nc()
    return _NC_CACHE


def _prep_inputs(x, weight_mu, weight_logvar, bias_mu, bias_logvar, eps_w, eps_b):
    """Host-side prep: shard eps over batch, pack the small params."""
    x = np.asarray(x, np.float32)
    weight_mu = np.asarray(weight_mu, np.float32)
    weight_logvar = np.asarray(weight_logvar, np.float32)
    bias_mu = np.asarray(bias_mu, np.float32)
    bias_logvar = np.asarray(bias_logvar, np.float32)
    eps_w = np.asarray(eps_w, np.float32)
    eps_b = np.asarray(eps_b, np.float32)

    std = np.exp(0.5 * weight_logvar)                  # (OUT, IN)
    bstd = np.exp(0.5 * bias_logvar)                   # (OUT,)
    stdT = np.ascontiguousarray(std.T).astype(np.float16)   # (IN, OUT)
    muT16 = np.ascontiguousarray(weight_mu.T).astype(np.float16)

    # srepT[p, blk(h_i, s, h_o) + c] = stdT[h_i*128+p, h_o*128+c]
    srepT = np.zeros((128, SREP_W), np.float16)
    for h_i in range(2):
        for s in range(S):
            for h_o in range(2):
                off = _blk(h_i, s, h_o)
                srepT[:, off:off + 128] = stdT[h_i * 128:(h_i + 1) * 128,
                                               h_o * 128:(h_o + 1) * 128]

    # muT packed as [p(i_l), h_i*OUT + o]
    muT = np.zeros((128, 2 * OUT), np.float16)
    for h_i in range(2):
        muT[:, h_i * OUT:(h_i + 1) * OUT] = muT16[h_i * 128:(h_i + 1) * 128, :]

    ident = np.eye(128, dtype=np.float16)

    in_maps = []
    for cix in range(N_CORES):
        sl = slice(cix * B_CORE, (cix + 1) * B_CORE)
        x_c = x[sl]                                     # (B_CORE, IN)
        xTc = np.ascontiguousarray(x_c.T).astype(np.float16)  # (IN, B_CORE)
        xT = np.zeros((128, 2 * B_CORE), np.float16)
        for h_i in range(2):
            xT[:, h_i * B_CORE:(h_i + 1) * B_CORE] = \
                xTc[h_i * 128:(h_i + 1) * 128, :]

        # biasesT[p, h_o*B_CORE + b] = bias_mu[o] + bstd[o]*eps_b[b, o],
        # o = h_o*128 + p
        bT = bias_mu[:, None] + bstd[:, None] * eps_b[sl].T  # (OUT, B_CORE)
        biasesT = np.zeros((128, 2 * B_CORE), np.float32)
        for h_o in range(2):
            biasesT[:, h_o * B_CORE:(h_o + 1) * B_CORE] = \
                bT[h_o * 128:(h_o + 1) * 128, :]

        in_maps.append({
            "eps": eps_w[sl],
            "srepT": srepT,
            "xT": xT,
            "muT": muT,
            "biasesT": biasesT,
            "ident": ident,
        })
    return in_maps


def run(trace=False, **inputs):
    nc = _get_nc()
    in_maps = _prep_inputs(**inputs)
    res = run_bass_kernel_spmd(nc, in_maps, list(range(N_CORES)), trace=trace)
    out = np.concatenate(
        [np.asarray(res.results[c]["out"]).T for c in range(N_CORES)], axis=0)
    return np.ascontiguousarray(out, np.float32), res


def kernel(**inputs) -> np.ndarray:
    out, _ = run(trace=False, **inputs)
    return out


# revision 5
# speedup vs baseline: 2.7413x; 1.2307x over previous
"""Trainium2 Bass kernel for BayesianLinear sampling (B=2048, OUT=IN=256).

out[b,o] = sum_i (mu[o,i] + std[o,i]*eps_w[b,o,i]) * x[b,i]
         + bias_mu[o] + bias_std[o]*eps_b[b,o]

Data-parallel over batch across 8 NeuronCores (256 samples each).

Per-core pipeline (out is produced transposed as (o, b); the host
transposes back):

  eps load   -- split across three DMA queues so the transfers overlap:
               even 16-sample chunks stream through the gpsimd (SWDGE)
               queue with an in-flight f32->f16 cast; odd chunks come in
               as raw f32 via the SP and ACT HWDGE queues (one o-half
               each) and are cast to f16 by the Scalar engine.
  PE         -- per 128x128 block, transpose eps (o,i)->(i,o) into an
               f16 PSUM slab (is_transpose matmul against an f16
               identity, 1 cycle/row).
  DVE        -- one tensor_tensor per 4-sample group: Q = epsT * stdT
               (PSUM f16 -> SBUF f16, 2x perf mode).  This doubles as
               the PSUM->SBUF copy.
  PE         -- per (sample, i-half, o-half): n=1 matmul with the Q
               block as the stationary operand and the sample's x
               column as the moving operand, accumulating
               outT[o, b] = sum_i Q[i,o] x[b,i] into PSUM.  The
               mu @ x^T base term is 4 more matmuls into a second PSUM
               tile.
  epilogue   -- DVE adds base + (bias_mu + bias_std*eps_b)^T (host
               precomputed) and the result is DMA'd out as (o, b).

The small parameters (mu, std=exp(0.5*logvar), x, biases) are packed on
the host; eps_w is shipped to the device untouched.
"""

import sys

sys.path.insert(0, "/opt/trn_rl_repo")

import numpy as np

import concourse.bass as bass
import concourse.bacc as bacc
import concourse.mybir as mybir
from concourse import tile
from concourse.bass_utils import run_bass_kernel_spmd

N_CORES = 8
B, OUT, IN = 2048, 256, 256
B_CORE = B // N_CORES          # 256 samples per core
SD = 16                        # samples per DMA chunk
S = 4                          # samples per transpose/fold group
NCHUNK = B_CORE // SD          # 16 chunks
F32 = mybir.dt.float32
F16 = mybir.dt.float16
MULT = mybir.AluOpType.mult
ADD = mybir.AluOpType.add

SREP_W = S * 2 * 2 * 128       # 2048: (h_i, s, h_o) blocks of 128


def _blk(h_i, s, h_o):
    """Free-dim offset of block (h_i, s, h_o) in a Q / QT slab."""
    return ((h_i * S + s) * 2 + h_o) * 128


def _build_nc():
    nc = bacc.Bacc(trn_type="TRN2")

    eps = nc.declare_dram_parameter("eps", [B_CORE, OUT, IN], F32, isOutput=False)
    srepT = nc.declare_dram_parameter("srepT", [128, SREP_W], F16, isOutput=False)
    xT = nc.declare_dram_parameter("xT", [128, 2 * B_CORE], F16, isOutput=False)
    muT = nc.declare_dram_parameter("muT", [128, 2 * OUT], F16, isOutput=False)
    biasesT = nc.declare_dram_parameter("biasesT", [128, 2 * B_CORE], F32,
                                        isOutput=False)
    ident = nc.declare_dram_parameter("ident", [128, 128], F16, isOutput=False)
    out = nc.declare_dram_parameter("out", [OUT, B_CORE], F32, isOutput=True)

    eps3 = eps.rearrange("b o i -> o b i")  # partition = o view for DMA

    with tile.TileContext(nc) as tc:
        with tc.tile_pool(name="const", bufs=1) as cpool:
            srepT_sb = cpool.tile([128, SREP_W], F16, tag="srepT", name="srepTsb")
            xT_sb = cpool.tile([128, 2 * B_CORE], F16, tag="xT", name="xTsb")
            muT_sb = cpool.tile([128, 2 * OUT], F16, tag="muT", name="muTsb")
            bias_sb = cpool.tile([128, 2 * B_CORE], F32, tag="biasesT",
                                 name="biassb")
            id_sb = cpool.tile([128, 128], F16, tag="ident", name="idsb")
            out_sb = [cpool.tile([128, B_CORE], F32, tag=f"osb{h}",
                                 name=f"osb{h}") for h in range(2)]
            tmp_sb = [cpool.tile([128, B_CORE], F32, tag=f"tsb{h}",
                                 name=f"tsb{h}") for h in range(2)]

            nc.sync.dma_start(out=srepT_sb[:], in_=srepT[:])
            nc.sync.dma_start(out=xT_sb[:], in_=xT[:])
            nc.sync.dma_start(out=muT_sb[:], in_=muT[:])
            nc.sync.dma_start(out=bias_sb[:], in_=biasesT[:])
            nc.sync.dma_start(out=id_sb[:], in_=ident[:])

            with (
                tc.tile_pool(name="ef", bufs=4) as efpool,
                tc.tile_pool(name="e32", bufs=2) as e32pool,
                tc.tile_pool(name="qt", bufs=2, space="PSUM") as qtpool,
                tc.tile_pool(name="q", bufs=3) as qpool,
                tc.tile_pool(name="acc", bufs=1, space="PSUM") as accpool,
            ):
                acc = [accpool.tile([128, B_CORE], F32, tag=f"acc{h}",
                                    name=f"acc{h}") for h in range(2)]
                pbase = [accpool.tile([128, B_CORE], F32, tag=f"pb{h}",
                                      name=f"pb{h}") for h in range(2)]

                # base term: pbase[h_o][o_l, b] = sum_i mu[o,i] x[b,i]
                for h_o in range(2):
                    for h_i in range(2):
                        nc.tensor.matmul(
                            pbase[h_o][:],
                            muT_sb[:, h_i * OUT + h_o * 128:
                                   h_i * OUT + (h_o + 1) * 128],
                            xT_sb[:, h_i * B_CORE:(h_i + 1) * B_CORE],
                            start=(h_i == 0), stop=(h_i == 1),
                        )

                for c in range(NCHUNK):
                    ef = efpool.tile([128, 2 * SD * IN], F16, tag="ef",
                                     name="ef")
                    # pool-path: even chunks plus the last one (short tail);
                    # hwdge-path: odd chunks 1..13
                    if c % 2 == 0 or c == NCHUNK - 1:
                        # SWDGE queue: in-flight f32 -> f16 cast
                        for h_o in range(2):
                            nc.gpsimd.dma_start(
                                out=ef[:, h_o * SD * IN:(h_o + 1) * SD * IN]
                                .rearrange("p (s i) -> p s i", i=IN),
                                in_=eps3[h_o * 128:(h_o + 1) * 128,
                                         c * SD:(c + 1) * SD, :],
                            )
                    else:
                        # HWDGE queues (SP + ACT): raw f32, then ACT casts
                        e32 = e32pool.tile([128, 2 * SD * IN], F32, tag="e32",
                                           name="e32")
                        for h_o, eng in ((0, nc.sync), (1, nc.scalar)):
                            eng.dma_start(
                                out=e32[:, h_o * SD * IN:(h_o + 1) * SD * IN]
                                .rearrange("p (s i) -> p s i", i=IN),
                                in_=eps3[h_o * 128:(h_o + 1) * 128,
                                         c * SD:(c + 1) * SD, :],
                            )
                        # cast halves on different engines so they overlap
                        nc.scalar.copy(
                            out=ef[:, 0:SD * IN],
                            in_=e32[:, 0:SD * IN],
                        )
                        nc.gpsimd.tensor_copy(
                            ef[:, SD * IN:2 * SD * IN],
                            e32[:, SD * IN:2 * SD * IN],
                        )

                    for gs in range(SD // S):
                        qt = qtpool.tile([128, SREP_W], F16, tag="qt",
                                         name="qt")
                        for h_o in range(2):
                            for s in range(S):
                                for h_i in range(2):
                                    nc.tensor.transpose(
                                        qt[:, _blk(h_i, s, h_o):
                                           _blk(h_i, s, h_o) + 128],
                                        ef[:, (h_o * SD + gs * S + s) * IN
                                           + h_i * 128:
                                           (h_o * SD + gs * S + s) * IN
                                           + h_i * 128 + 128],
                                        id_sb[:],
                                    )
                        q = qpool.tile([128, SREP_W], F16, tag="q", name="q")
                        nc.vector.tensor_tensor(out=q[:], in0=qt[:],
                                                in1=srepT_sb[:], op=MULT)
                        for s in range(S):
                            b = c * SD + gs * S + s
                            for h_o in range(2):
                                for h_i in range(2):
                                    nc.tensor.matmul(
                                        acc[h_o][:, b:b + 1],
                                        q[:, _blk(h_i, s, h_o):
                                          _blk(h_i, s, h_o) + 128],
                                        xT_sb[:, h_i * B_CORE + b:
                                              h_i * B_CORE + b + 1],
                                        start=(h_i == 0), stop=(h_i == 1),
                                    )

                # epilogue: out_sb = acc + biasesT + pbase  (each DVE op
                # touches at most one PSUM operand)
                for h_o in range(2):
                    nc.vector.tensor_tensor(
                        out=tmp_sb[h_o][:], in0=acc[h_o][:],
                        in1=bias_sb[:, h_o * B_CORE:(h_o + 1) * B_CORE],
                        op=ADD)
                    nc.vector.tensor_tensor(
                        out=out_sb[h_o][:], in0=pbase[h_o][:],
                        in1=tmp_sb[h_o][:], op=ADD)
                    nc.sync.dma_start(
                        out=out[h_o * 128:(h_o + 1) * 128, :],
                        in_=out_sb[h_o][:])

    nc.compile()
    return nc


_NC_CACHE = None


def _get_nc():
    global _NC_CACHE
    if _NC_CACHE is None:
        _NC_CACHE = _build_nc()
    return _NC_CACHE


def _prep_inputs(x, weight_mu, weight_logvar, bias_mu, bias_logvar, eps_w, eps_b):
    """Host-side prep: shard eps over batch, pack the small params."""
    x = np.asarray(x, np.float32)
    weight_mu = np.asarray(weight_mu, np.float32)
    weight_logvar = np.asarray(weight_logvar, np.float32)
    bias_mu = np.asarray(bias_mu, np.float32)
    bias_logvar = np.asarray(bias_logvar, np.float32)
    eps_w = np.asarray(eps_w, np.float32)
    eps_b = np.asarray(eps_b, np.float32)

    std = np.exp(0.5 * weight_logvar)                  # (OUT, IN)
    bstd = np.exp(0.5 * bias_logvar)                   # (OUT,)
    stdT = np.ascontiguousarray(std.T).astype(np.float16)   # (IN, OUT)
    muT16 = np.ascontiguousarray(weight_mu.T).astype(np.float16)

    # srepT[p, blk(h_i, s, h_o) + c] = stdT[h_i*128+p, h_o*128+c]
    srepT = np.zeros((128, SREP_W), np.float16)
    for h_i in range(2):
        for s in range(S):
            for h_o in range(2):
                off = _blk(h_i, s, h_o)
                srepT[:, off:off + 128] = stdT[h_i * 128:(h_i + 1) * 128,
                                               h_o * 128:(h_o + 1) * 128]

    # muT packed as [p(i_l), h_i*OUT + o]
    muT = np.zeros((128, 2 * OUT), np.float16)
    for h_i in range(2):
        muT[:, h_i * OUT:(h_i + 1) * OUT] = muT16[h_i * 128:(h_i + 1) * 128, :]

    ident = np.eye(128, dtype=np.float16)

    in_maps = []
    for cix in range(N_CORES):
        sl = slice(cix * B_CORE, (cix + 1) * B_CORE)
        x_c = x[sl]                                     # (B_CORE, IN)
        xTc = np.ascontiguousarray(x_c.T).astype(np.float16)  # (IN, B_CORE)
        xT = np.zeros((128, 2 * B_CORE), np.float16)
        for h_i in range(2):
            xT[:, h_i * B_CORE:(h_i + 1) * B_CORE] = \
                xTc[h_i * 128:(h_i + 1) * 128, :]

        # biasesT[p, h_o*B_CORE + b] = bias_mu[o] + bstd[o]*eps_b[b, o],
        # o = h_o*128 + p
        bT = bias_mu[:, None] + bstd[:, None] * eps_b[sl].T  # (OUT, B_CORE)
        biasesT = np.zeros((128, 2 * B_CORE), np.float32)
        for h_o in range(2):
            biasesT[:, h_o * B_CORE:(h_o + 1) * B_CORE] = \
                bT[h_o * 128:(h_o + 1) * 128, :]

        in_maps.append({
            "eps": eps_w[sl],
            "srepT": srepT,
            "xT": xT,
            "muT": muT,
            "biasesT": biasesT,
            "ident": ident,
        })
    return in_maps


def run(trace=False, **inputs):
    nc = _get_nc()
    in_maps = _prep_inputs(**inputs)
    res = run_bass_kernel_spmd(nc, in_maps, list(range(N_CORES)), trace=trace)
    out = np.concatenate(
        [np.asarray(res.results[c]["out"]).T for c in range(N_CORES)], axis=0)
    return np.ascontiguousarray(out, np.float32), res


def kernel(**inputs) -> np.ndarray:
    out, _ = run(trace=False, **inputs)
    return out


# revision 6
# speedup vs baseline: 2.8870x; 1.0532x over previous
"""Trainium2 Bass kernel for BayesianLinear sampling (B=2048, OUT=IN=256).

out[b,o] = sum_i (mu[o,i] + std[o,i]*eps_w[b,o,i]) * x[b,i]
         + bias_mu[o] + bias_std[o]*eps_b[b,o]

Data-parallel over batch across 8 NeuronCores (256 samples each).

Per-core pipeline (out is produced transposed as (o, b); the host
transposes back):

  eps load   -- split across three DMA queues so the transfers overlap:
               even 16-sample chunks stream through the gpsimd (SWDGE)
               queue with an in-flight f32->f16 cast; odd chunks come in
               as raw f32 via the SP and ACT HWDGE queues (one o-half
               each) and are cast to f16 by the Scalar engine.
  PE         -- per 128x128 block, transpose eps (o,i)->(i,o) into an
               f16 PSUM slab (is_transpose matmul against an f16
               identity, 1 cycle/row).
  DVE        -- one tensor_tensor per 4-sample group: Q = epsT * stdT
               (PSUM f16 -> SBUF f16, 2x perf mode).  This doubles as
               the PSUM->SBUF copy.
  PE         -- per (sample, i-half, o-half): n=1 matmul with the Q
               block as the stationary operand and the sample's x
               column as the moving operand, accumulating
               outT[o, b] = sum_i Q[i,o] x[b,i] into PSUM.  The
               mu @ x^T base term is 4 more matmuls into a second PSUM
               tile.
  epilogue   -- DVE adds base + (bias_mu + bias_std*eps_b)^T (host
               precomputed) and the result is DMA'd out as (o, b).

The small parameters (mu, std=exp(0.5*logvar), x, biases) are packed on
the host; eps_w is shipped to the device untouched.
"""

import sys

sys.path.insert(0, "/opt/trn_rl_repo")

import numpy as np

import concourse.bass as bass
import concourse.bacc as bacc
import concourse.mybir as mybir
from concourse import tile
from concourse.bass_utils import run_bass_kernel_spmd

N_CORES = 8
B, OUT, IN = 2048, 256, 256
B_CORE = B // N_CORES          # 256 samples per core
SD = 16                        # samples per DMA chunk
S = 4                          # samples per transpose/fold group
NCHUNK = B_CORE // SD          # 16 chunks
F32 = mybir.dt.float32
F16 = mybir.dt.float16
MULT = mybir.AluOpType.mult
ADD = mybir.AluOpType.add

SREP_W = S * 2 * 2 * 128       # 2048: (h_i, s, h_o) blocks of 128


def _blk(h_i, s, h_o):
    """Free-dim offset of block (h_i, s, h_o) in a Q / QT slab."""
    return ((h_i * S + s) * 2 + h_o) * 128


def _build_nc():
    nc = bacc.Bacc(trn_type="TRN2")

    eps = nc.declare_dram_parameter("eps", [B_CORE, OUT, IN], F32, isOutput=False)
    srepT = nc.declare_dram_parameter("srepT", [128, SREP_W], F16, isOutput=False)
    xT = nc.declare_dram_parameter("xT", [128, 2 * B_CORE], F16, isOutput=False)
    muT = nc.declare_dram_parameter("muT", [128, 2 * OUT], F16, isOutput=False)
    biasesT = nc.declare_dram_parameter("biasesT", [128, 2 * B_CORE], F32,
                                        isOutput=False)
    ident = nc.declare_dram_parameter("ident", [128, 128], F16, isOutput=False)
    out = nc.declare_dram_parameter("out", [OUT, B_CORE], F32, isOutput=True)

    eps3 = eps.rearrange("b o i -> o b i")  # partition = o view for DMA

    with tile.TileContext(nc) as tc:
        with tc.tile_pool(name="const", bufs=1) as cpool:
            srepT_sb = cpool.tile([128, SREP_W], F16, tag="srepT", name="srepTsb")
            xT_sb = cpool.tile([128, 2 * B_CORE], F16, tag="xT", name="xTsb")
            muT_sb = cpool.tile([128, 2 * OUT], F16, tag="muT", name="muTsb")
            bias_sb = cpool.tile([128, 2 * B_CORE], F32, tag="biasesT",
                                 name="biassb")
            id_sb = cpool.tile([128, 128], F16, tag="ident", name="idsb")
            out_sb = [cpool.tile([128, B_CORE], F32, tag=f"osb{h}",
                                 name=f"osb{h}") for h in range(2)]
            tmp_sb = [cpool.tile([128, B_CORE], F32, tag=f"tsb{h}",
                                 name=f"tsb{h}") for h in range(2)]

            nc.sync.dma_start(out=srepT_sb[:], in_=srepT[:])
            nc.sync.dma_start(out=xT_sb[:], in_=xT[:])
            nc.sync.dma_start(out=muT_sb[:], in_=muT[:])
            nc.sync.dma_start(out=bias_sb[:], in_=biasesT[:])
            nc.sync.dma_start(out=id_sb[:], in_=ident[:])

            with (
                tc.tile_pool(name="ef", bufs=4) as efpool,
                tc.tile_pool(name="e32", bufs=2) as e32pool,
                tc.tile_pool(name="qt", bufs=2, space="PSUM") as qtpool,
                tc.tile_pool(name="q", bufs=3) as qpool,
                tc.tile_pool(name="acc", bufs=1, space="PSUM") as accpool,
            ):
                acc = [accpool.tile([128, B_CORE], F32, tag=f"acc{h}",
                                    name=f"acc{h}") for h in range(2)]
                pbase = [accpool.tile([128, B_CORE], F32, tag=f"pb{h}",
                                      name=f"pb{h}") for h in range(2)]

                # base term: pbase[h_o][o_l, b] = sum_i mu[o,i] x[b,i]
                for h_o in range(2):
                    for h_i in range(2):
                        nc.tensor.matmul(
                            pbase[h_o][:],
                            muT_sb[:, h_i * OUT + h_o * 128:
                                   h_i * OUT + (h_o + 1) * 128],
                            xT_sb[:, h_i * B_CORE:(h_i + 1) * B_CORE],
                            start=(h_i == 0), stop=(h_i == 1),
                        )

                # pool-path: even chunks plus the last one (short tail);
                # hwdge-path: odd chunks 1..13.  Front-load pool chunks in
                # program order so DVE has work while the first hwdge
                # chunk's longer DMA->cast chain fills.
                order = [0, 2, 1, 4, 3, 6, 5, 8, 7, 10, 9, 12, 11, 14, 13, 15]
                for c in order:
                    ef = efpool.tile([128, 2 * SD * IN], F16, tag="ef",
                                     name="ef")
                    if c % 2 == 0 or c == NCHUNK - 1:
                        # SWDGE queue: in-flight f32 -> f16 cast
                        for h_o in range(2):
                            nc.gpsimd.dma_start(
                                out=ef[:, h_o * SD * IN:(h_o + 1) * SD * IN]
                                .rearrange("p (s i) -> p s i", i=IN),
                                in_=eps3[h_o * 128:(h_o + 1) * 128,
                                         c * SD:(c + 1) * SD, :],
                            )
                    else:
                        # HWDGE queues (SP + ACT): raw f32, then ACT casts
                        e32 = e32pool.tile([128, 2 * SD * IN], F32, tag="e32",
                                           name="e32")
                        for h_o, eng in ((0, nc.sync), (1, nc.scalar)):
                            eng.dma_start(
                                out=e32[:, h_o * SD * IN:(h_o + 1) * SD * IN]
                                .rearrange("p (s i) -> p s i", i=IN),
                                in_=eps3[h_o * 128:(h_o + 1) * 128,
                                         c * SD:(c + 1) * SD, :],
                            )
                        # cast halves on different engines so they overlap
                        nc.scalar.copy(
                            out=ef[:, 0:SD * IN],
                            in_=e32[:, 0:SD * IN],
                        )
                        nc.gpsimd.tensor_copy(
                            ef[:, SD * IN:2 * SD * IN],
                            e32[:, SD * IN:2 * SD * IN],
                        )

                    for gs in range(SD // S):
                        qt = qtpool.tile([128, SREP_W], F16, tag="qt",
                                         name="qt")
                        for h_o in range(2):
                            for s in range(S):
                                for h_i in range(2):
                                    nc.tensor.transpose(
                                        qt[:, _blk(h_i, s, h_o):
                                           _blk(h_i, s, h_o) + 128],
                                        ef[:, (h_o * SD + gs * S + s) * IN
                                           + h_i * 128:
                                           (h_o * SD + gs * S + s) * IN
                                           + h_i * 128 + 128],
                                        id_sb[:],
                                    )
                        q = qpool.tile([128, SREP_W], F16, tag="q", name="q")
                        nc.vector.tensor_tensor(out=q[:], in0=qt[:],
                                                in1=srepT_sb[:], op=MULT)
                        for s in range(S):
                            b = c * SD + gs * S + s
                            for h_o in range(2):
                                for h_i in range(2):
                                    nc.tensor.matmul(
                                        acc[h_o][:, b:b + 1],
                                        q[:, _blk(h_i, s, h_o):
                                          _blk(h_i, s, h_o) + 128],
                                        xT_sb[:, h_i * B_CORE + b:
                                              h_i * B_CORE + b + 1],
                                        start=(h_i == 0), stop=(h_i == 1),
                                    )

                # epilogue: out_sb = acc + biasesT + pbase  (each DVE op
                # touches at most one PSUM operand)
                for h_o in range(2):
                    nc.vector.tensor_tensor(
                        out=tmp_sb[h_o][:], in0=acc[h_o][:],
                        in1=bias_sb[:, h_o * B_CORE:(h_o + 1) * B_CORE],
                        op=ADD)
                    nc.vector.tensor_tensor(
                        out=out_sb[h_o][:], in0=pbase[h_o][:],
                        in1=tmp_sb[h_o][:], op=ADD)
                    nc.sync.dma_start(
                        out=out[h_o * 128:(h_o + 1) * 128, :],
                        in_=out_sb[h_o][:])

    nc.compile()
    return nc


_NC_CACHE = None


def _get_nc():
    global _NC_CACHE
    if _NC_CACHE is None:
        _NC_CACHE = _build_nc()
    return _NC_CACHE


def _prep_inputs(x, weight_mu, weight_logvar, bias_mu, bias_logvar, eps_w, eps_b):
    """Host-side prep: shard eps over batch, pack the small params."""
    x = np.asarray(x, np.float32)
    weight_mu = np.asarray(weight_mu, np.float32)
    weight_logvar = np.asarray(weight_logvar, np.float32)
    bias_mu = np.asarray(bias_mu, np.float32)
    bias_logvar = np.asarray(bias_logvar, np.float32)
    eps_w = np.asarray(eps_w, np.float32)
    eps_b = np.asarray(eps_b, np.float32)

    std = np.exp(0.5 * weight_logvar)                  # (OUT, IN)
    bstd = np.exp(0.5 * bias_logvar)                   # (OUT,)
    stdT = np.ascontiguousarray(std.T).astype(np.float16)   # (IN, OUT)
    muT16 = np.ascontiguousarray(weight_mu.T).astype(np.float16)

    # srepT[p, blk(h_i, s, h_o) + c] = stdT[h_i*128+p, h_o*128+c]
    srepT = np.zeros((128, SREP_W), np.float16)
    for h_i in range(2):
        for s in range(S):
            for h_o in range(2):
                off = _blk(h_i, s, h_o)
                srepT[:, off:off + 128] = stdT[h_i * 128:(h_i + 1) * 128,
                                               h_o * 128:(h_o + 1) * 128]

    # muT packed as [p(i_l), h_i*OUT + o]
    muT = np.zeros((128, 2 * OUT), np.float16)
    for h_i in range(2):
        muT[:, h_i * OUT:(h_i + 1) * OUT] = muT16[h_i * 128:(h_i + 1) * 128, :]

    ident = np.eye(128, dtype=np.float16)

    in_maps = []
    for cix in range(N_CORES):
        sl = slice(cix * B_CORE, (cix + 1) * B_CORE)
        x_c = x[sl]                                     # (B_CORE, IN)
        xTc = np.ascontiguousarray(x_c.T).astype(np.float16)  # (IN, B_CORE)
        xT = np.zeros((128, 2 * B_CORE), np.float16)
        for h_i in range(2):
            xT[:, h_i * B_CORE:(h_i + 1) * B_CORE] = \
                xTc[h_i * 128:(h_i + 1) * 128, :]

        # biasesT[p, h_o*B_CORE + b] = bias_mu[o] + bstd[o]*eps_b[b, o],
        # o = h_o*128 + p
        bT = bias_mu[:, None] + bstd[:, None] * eps_b[sl].T  # (OUT, B_CORE)
        biasesT = np.zeros((128, 2 * B_CORE), np.float32)
        for h_o in range(2):
            biasesT[:, h_o * B_CORE:(h_o + 1) * B_CORE] = \
                bT[h_o * 128:(h_o + 1) * 128, :]

        in_maps.append({
            "eps": eps_w[sl],
            "srepT": srepT,
            "xT": xT,
            "muT": muT,
            "biasesT": biasesT,
            "ident": ident,
        })
    return in_maps


def run(trace=False, **inputs):
    nc = _get_nc()
    in_maps = _prep_inputs(**inputs)
    res = run_bass_kernel_spmd(nc, in_maps, list(range(N_CORES)), trace=trace)
    out = np.concatenate(
        [np.asarray(res.results[c]["out"]).T for c in range(N_CORES)], axis=0)
    return np.ascontiguousarray(out, np.float32), res


def kernel(**inputs) -> np.ndarray:
    out, _ = run(trace=False, **inputs)
    return out
